# revision 15
# baseline (speedup 1.0000x reference)
"""LoRA layer kernel for Trainium2 (8 NeuronCores, data-parallel over rows).

Computes out = ((x @ V^T) * S) @ U^T * scaling  (scaling = alpha/rank = 1.0)
for x [4, 2048, 4096], U [4096, 32], S [32], V [32, 4096], all fp32.

Sharding: batch*seq rows (8192) split evenly across the 8 cores; the tiny
LoRA factors are replicated. All layout prep happens on the host:
  - x is cast to bf16 and pre-transposed/tiled to [chunk, p, ft, row] so the
    device reads features-on-partitions directly (no on-device transposes)
  - V is cast to bf16, pre-tiled to [p, ft, rank]
  - U is scaled by S*scaling, transposed, cast to bf16
Output is written bf16 (halves the store traffic) and upcast to fp32 on the
host; bf16 keeps max rel err ~5e-3 against the fp32 reference.

Per core (1024 rows, 2 chunks of 512):
  - input DMAs on the ACT HWDGE ring in 1 MiB ft-quarters, output DMAs on
    the SP ring, so stores interleave with loads at the SDMA engines
  - mm1: hT[32, 512] += vsT[:, ft, :]^T @ xt[:, ft, :] accumulated over the
    32 feature tiles in one PSUM bank (bf16); per-quarter DMA deps let the
    PE start after the first 1 MiB lands
  - hT copied PSUM->SBUF as bf16 (DVE)
  - mm2: per row tile, one 8-matmul group reusing the same stationary
    hT-slice (one LDWEIGHTS per 8 matmuls); chunk-0 groups are interleaved
    into mm1(1)'s feature loop so output DMA overlaps the x1 load;
    PSUM->SBUF copies split DVE/ScalarE 50/50 with bf16 downcast
  - per-row-tile 1 MiB DMA stores issued right after each group's copies
Roofline: ~16.4 MiB HBM traffic per core at ~360-425 GB/s => ~42-47 us;
PE ~33 us hidden under DMA. No collectives needed.
"""

import sys

for _p in ("/root/.axon_site/_ro/trn_rl_repo", "/opt/trn_rl_repo"):
    if _p not in sys.path:
        sys.path.append(_p)

import ml_dtypes
import numpy as np

import concourse.bass as bass
from concourse import mybir
from concourse.bass_utils import run_bass_kernel_spmd
from concourse.tile import TileContext

F32 = mybir.dt.float32
BF16 = mybir.dt.bfloat16
NP_BF16 = ml_dtypes.bfloat16

P = 128
ROWS = 1024  # per-core row shard
FEAT = 4096
RANK = 32
SCALING = 1.0  # alpha / max_rank = 32 / 32
FT = FEAT // P  # 32 feature tiles
CHUNK = 512  # rows per pipeline chunk
CHUNK_TILES = CHUNK // P  # 4
N_CHUNKS = ROWS // CHUNK  # 2
NQ = 4  # input DMA quarters per chunk
FQ = FT // NQ  # 8 feature tiles per quarter
OC = FEAT // 512  # 8 output column chunks per row tile
N_CORES = 8


def _split_multiwaits(nc) -> None:
    # Workaround for this container's walrus: engine instructions with >=2
    # sem waits fail codegen ("Too many sync wait commands"). Hoist all but
    # the last wait onto single-wait NoOps inserted just before, same engine.
    for f in nc.m.functions:
        for bb in f.blocks:
            out = []
            changed = False
            for inst in bb.instructions:
                si = inst.sync_info
                waits = list(si.on_wait) if (si is not None and si.on_wait) else []
                if len(waits) > 1:
                    changed = True
                    for w in waits[:-1]:
                        nop = mybir.InstNoOp(name=f"splitw-{nc.next_id()}")
                        nop.engine = inst.engine
                        nop.sync_info = mybir.SyncInfo(on_wait=[w], on_update=[])
                        nc.register_instruction(nop)
                        out.append(nop)
                    si.on_wait = [waits[-1]]
                out.append(inst)
            if changed:
                bb.instructions = out


class _PatchedTileContext(TileContext):
    def _drain_and_barrier(self, tick_clock, wait_clock):
        super()._drain_and_barrier(tick_clock, wait_clock)
        _split_multiwaits(self.nc)


def build_nc() -> bass.Bass:
    nc = bass.Bass(trn_type="TRN2", target_bir_lowering=False, name="lora")
    # xt host layout: [chunk, p, ft, row-in-chunk]; ft-quarter slices are
    # 8 KiB-per-partition contiguous DMAs
    xt_d = nc.dram_tensor("xt", [N_CHUNKS, P, FT * CHUNK], BF16, kind="ExternalInput")
    vt_d = nc.dram_tensor("vt", [P, FT * RANK], BF16, kind="ExternalInput")
    ut_d = nc.dram_tensor("ut", [RANK, FEAT], BF16, kind="ExternalInput")
    out_d = nc.dram_tensor("out", [ROWS, FEAT], BF16, kind="ExternalOutput")

    with _PatchedTileContext(nc) as tc:
        with (
            tc.tile_pool(name="consts", bufs=1) as consts,
            tc.tile_pool(name="xin", bufs=N_CHUNKS) as x_pool,
            tc.tile_pool(name="hts", bufs=2) as h_pool,
            tc.tile_pool(name="outs", bufs=2) as out_pool,
            tc.tile_pool(name="ps_h", bufs=2, space="PSUM") as psum_h,
            tc.tile_pool(name="ps_o", bufs=5, space="PSUM") as psum_o,
            tc.tile_pool(name="ps_w", bufs=1, space="PSUM") as psum_w,
        ):
            # All DMAs ride the SP (sync) HWDGE ring: the FIFO gives input
            # loads strict priority over output stores, which is what we want
            # — the back half is production(copy)-limited anyway, and early
            # input completion unblocks mm1(1). Issue order = need order:
            # vt, x0 quarters, ut, x1 quarters, stores as produced.
            vsT = consts.tile([P, FT, RANK], BF16)
            nc.sync.dma_start(vsT, vt_d[:, :].rearrange("p (f r) -> p f r", r=RANK))

            x_tiles = []
            srcs = []
            for c in range(N_CHUNKS):
                xt = x_pool.tile([P, FT, CHUNK], BF16, tag="x")
                x_tiles.append(xt)
                srcs.append(xt_d[c, :, :].rearrange("p (f r) -> p f r", r=CHUNK))
            for q in range(NQ):
                nc.sync.dma_start(
                    x_tiles[0][:, q * FQ : (q + 1) * FQ, :],
                    srcs[0][:, q * FQ : (q + 1) * FQ, :],
                )
            usT = consts.tile([RANK, FEAT], BF16)
            nc.sync.dma_start(usT, ut_d[:, :])
            for q in range(NQ):
                nc.sync.dma_start(
                    x_tiles[1][:, q * FQ : (q + 1) * FQ, :],
                    srcs[1][:, q * FQ : (q + 1) * FQ, :],
                )

            # A few dummy matmuls on zeroed scratch bridge the PE from engine
            # boot until the first x quarter lands, so the HAM activity
            # window sees continuous busy-ness and lifts the clock to 2.4GHz
            # during mm1(0)'s first quarter instead of halfway through.
            warm_sb = consts.tile([P, CHUNK], BF16)
            nc.vector.memset(warm_sb, 0.0)

            def emit_dummy_mm(n):
                for _ in range(n):
                    ps_w = psum_w.tile([P, CHUNK], F32, tag="w")
                    nc.tensor.matmul(
                        ps_w,
                        warm_sb[:, :P],
                        warm_sb,
                        start=True,
                        stop=True,
                        skip_group_check=True,
                    )

            emit_dummy_mm(3)

            def emit_mm1_ft(c, ps_h, ft):
                nc.tensor.matmul(
                    ps_h,
                    vsT[:, ft, :],
                    x_tiles[c][:, ft, :],
                    start=(ft == 0),
                    stop=(ft == FT - 1),
                    skip_group_check=True,
                )

            def emit_mm2_rt(hT, out_sb, ci, rt, fill=0):
                # one row tile: 8 matmuls sharing the same stationary
                # hT-slice, then copies (DVE/ACT alternating) and the store.
                # `fill` dummy matmuls after the group keep the PE's HAM
                # activity up through the copy-paced drain (otherwise it
                # re-throttles to 1.2 GHz and the matmuls become the wall).
                pss = []
                for oc in range(OC):
                    ps_o = psum_o.tile([P, 512], F32, tag="po")
                    nc.tensor.matmul(
                        ps_o,
                        hT[:, rt * P : (rt + 1) * P],
                        usT[:, oc * 512 : (oc + 1) * 512],
                        start=True,
                        stop=True,
                        skip_group_check=True,
                    )
                    pss.append(ps_o)
                emit_dummy_mm(fill)
                for oc, ps_o in enumerate(pss):
                    dst = out_sb[:, rt, oc * 512 : (oc + 1) * 512]
                    if oc % 2 == 0:
                        nc.vector.tensor_copy(out=dst, in_=ps_o)
                    else:
                        nc.scalar.copy(out=dst, in_=ps_o)
                r0 = ci * CHUNK + rt * P
                nc.sync.dma_start(out_d[r0 : r0 + P, :], out_sb[:, rt, :])

            # chunk 0: mm1 over 32 feature tiles
            ps_h0 = psum_h.tile([RANK, CHUNK], F32, tag="h")
            for ft in range(FT):
                emit_mm1_ft(0, ps_h0, ft)
            hT0 = h_pool.tile([RANK, CHUNK], BF16, tag="hT")
            nc.vector.tensor_copy(out=hT0, in_=ps_h0)
            out_sb0 = out_pool.tile([P, CHUNK_TILES, FEAT], BF16, tag="out")

            # alternate mm2(0) row-tile groups with mm1(1) quarter groups:
            # mm2(0) is ready immediately (keeps the copy engines fed) while
            # mm1(1) consumes x1 quarters as they land
            ps_h1 = psum_h.tile([RANK, CHUNK], F32, tag="h")
            for q in range(NQ):
                emit_mm2_rt(hT0, out_sb0, 0, q)
                for ft in range(q * FQ, (q + 1) * FQ):
                    emit_mm1_ft(1, ps_h1, ft)
            hT1 = h_pool.tile([RANK, CHUNK], BF16, tag="hT")
            nc.vector.tensor_copy(out=hT1, in_=ps_h1)
            out_sb1 = out_pool.tile([P, CHUNK_TILES, FEAT], BF16, tag="out")
            for rt in range(CHUNK_TILES):
                emit_mm2_rt(hT1, out_sb1, 1, rt, fill=4 if rt < CHUNK_TILES - 1 else 0)
    return nc


_NC_CACHE = None


def _get_nc():
    global _NC_CACHE
    if _NC_CACHE is None:
        _NC_CACHE = build_nc()
    return _NC_CACHE


def make_in_maps(x2, U, S, V):
    xb = np.ascontiguousarray(x2, dtype=np.float32).astype(NP_BF16)
    vb = np.ascontiguousarray(V, dtype=np.float32).astype(NP_BF16)
    # vt[p, ft, r] = V[r, ft*P + p]
    vt = np.ascontiguousarray(vb.reshape(RANK, FT, P).transpose(2, 1, 0)).reshape(
        P, FT * RANK
    )
    us = np.asarray(U, dtype=np.float32) * (
        np.asarray(S, dtype=np.float32)[None, :] * SCALING
    )
    ut = np.ascontiguousarray(us.T).astype(NP_BF16)
    maps = []
    for i in range(N_CORES):
        xs = xb[i * ROWS : (i + 1) * ROWS]
        # xt[c, p, ft, r] = xs[c*CHUNK + r, ft*P + p]
        xt = np.ascontiguousarray(
            xs.reshape(N_CHUNKS, CHUNK, FT, P).transpose(0, 3, 2, 1)
        ).reshape(N_CHUNKS, P, FT * CHUNK)
        maps.append({"xt": xt, "vt": vt, "ut": ut})
    return maps


def kernel(**inputs) -> np.ndarray:
    x = np.asarray(inputs["x"])
    U = inputs["U"]
    S = inputs["S"]
    V = inputs["V"]

    b, sq, feat = x.shape
    x2 = x.reshape(b * sq, feat)

    nc = _get_nc()
    in_maps = make_in_maps(x2, U, S, V)
    res = run_bass_kernel_spmd(nc, in_maps, core_ids=list(range(N_CORES)))
    out = np.concatenate([r["out"] for r in res.results], axis=0)
    return out.astype(np.float32).reshape(b, sq, feat)


# revision 16
# speedup vs baseline: 1.0218x; 1.0218x over previous
"""LoRA layer kernel for Trainium2 (8 NeuronCores, data-parallel over rows).

Computes out = ((x @ V^T) * S) @ U^T * scaling  (scaling = alpha/rank = 1.0)
for x [4, 2048, 4096], U [4096, 32], S [32], V [32, 4096], all fp32.

Sharding: batch*seq rows (8192) split evenly across the 8 cores; the tiny
LoRA factors are replicated. All layout prep happens on the host:
  - x is cast to bf16 and pre-transposed/tiled to [chunk, p, ft, row] so the
    device reads features-on-partitions directly (no on-device transposes)
  - V is cast to bf16, pre-tiled to [p, ft, rank]
  - U is scaled by S*scaling, transposed, cast to bf16
Output is written bf16 (halves the store traffic) and upcast to fp32 on the
host; bf16 keeps max rel err ~5e-3 against the fp32 reference.

Per core (1024 rows, 2 chunks of 512):
  - input DMAs on the ACT HWDGE ring in 1 MiB ft-quarters, output DMAs on
    the SP ring, so stores interleave with loads at the SDMA engines
  - mm1: hT[32, 512] += vsT[:, ft, :]^T @ xt[:, ft, :] accumulated over the
    32 feature tiles in one PSUM bank (bf16); per-quarter DMA deps let the
    PE start after the first 1 MiB lands
  - hT copied PSUM->SBUF as bf16 (DVE)
  - mm2: per row tile, one 8-matmul group reusing the same stationary
    hT-slice (one LDWEIGHTS per 8 matmuls); chunk-0 groups are interleaved
    into mm1(1)'s feature loop so output DMA overlaps the x1 load;
    PSUM->SBUF copies split DVE/ScalarE 50/50 with bf16 downcast
  - per-row-tile 1 MiB DMA stores issued right after each group's copies
Roofline: ~16.4 MiB HBM traffic per core at ~360-425 GB/s => ~42-47 us;
PE ~33 us hidden under DMA. No collectives needed.
"""

import sys

for _p in ("/root/.axon_site/_ro/trn_rl_repo", "/opt/trn_rl_repo"):
    if _p not in sys.path:
        sys.path.append(_p)

import ml_dtypes
import numpy as np

import concourse.bass as bass
from concourse import mybir
from concourse.bass_utils import run_bass_kernel_spmd
from concourse.tile import TileContext

F32 = mybir.dt.float32
BF16 = mybir.dt.bfloat16
NP_BF16 = ml_dtypes.bfloat16

P = 128
ROWS = 1024  # per-core row shard
FEAT = 4096
RANK = 32
SCALING = 1.0  # alpha / max_rank = 32 / 32
FT = FEAT // P  # 32 feature tiles
CHUNK = 512  # rows per pipeline chunk
CHUNK_TILES = CHUNK // P  # 4
N_CHUNKS = ROWS // CHUNK  # 2
NQ = 4  # input DMA quarters per chunk
FQ = FT // NQ  # 8 feature tiles per quarter
OC = FEAT // 512  # 8 output column chunks per row tile
N_CORES = 8


def _split_multiwaits(nc) -> None:
    # Workaround for this container's walrus: engine instructions with >=2
    # sem waits fail codegen ("Too many sync wait commands"). Hoist all but
    # the last wait onto single-wait NoOps inserted just before, same engine.
    for f in nc.m.functions:
        for bb in f.blocks:
            out = []
            changed = False
            for inst in bb.instructions:
                si = inst.sync_info
                waits = list(si.on_wait) if (si is not None and si.on_wait) else []
                if len(waits) > 1:
                    changed = True
                    for w in waits[:-1]:
                        nop = mybir.InstNoOp(name=f"splitw-{nc.next_id()}")
                        nop.engine = inst.engine
                        nop.sync_info = mybir.SyncInfo(on_wait=[w], on_update=[])
                        nc.register_instruction(nop)
                        out.append(nop)
                    si.on_wait = [waits[-1]]
                out.append(inst)
            if changed:
                bb.instructions = out


class _PatchedTileContext(TileContext):
    def _drain_and_barrier(self, tick_clock, wait_clock):
        super()._drain_and_barrier(tick_clock, wait_clock)
        _split_multiwaits(self.nc)


def build_nc() -> bass.Bass:
    nc = bass.Bass(trn_type="TRN2", target_bir_lowering=False, name="lora")
    # xt host layout: [chunk, p, ft, row-in-chunk]; ft-quarter slices are
    # 8 KiB-per-partition contiguous DMAs
    xt_d = nc.dram_tensor("xt", [N_CHUNKS, P, FT * CHUNK], BF16, kind="ExternalInput")
    vt_d = nc.dram_tensor("vt", [P, FT * RANK], BF16, kind="ExternalInput")
    ut_d = nc.dram_tensor("ut", [RANK, FEAT], BF16, kind="ExternalInput")
    out_d = nc.dram_tensor("out", [ROWS, FEAT], BF16, kind="ExternalOutput")

    with _PatchedTileContext(nc) as tc:
        with (
            tc.tile_pool(name="consts", bufs=1) as consts,
            tc.tile_pool(name="xin", bufs=N_CHUNKS) as x_pool,
            tc.tile_pool(name="hts", bufs=2) as h_pool,
            tc.tile_pool(name="outs", bufs=2) as out_pool,
            tc.tile_pool(name="ps_h", bufs=2, space="PSUM") as psum_h,
            tc.tile_pool(name="ps_o", bufs=5, space="PSUM") as psum_o,
            tc.tile_pool(name="ps_w", bufs=1, space="PSUM") as psum_w,
        ):
            # All DMAs ride the SP (sync) HWDGE ring: the FIFO gives input
            # loads strict priority over output stores, which is what we want
            # — the back half is production(copy)-limited anyway, and early
            # input completion unblocks mm1(1). Issue order = need order:
            # vt, x0 quarters, ut, x1 quarters, stores as produced.
            vsT = consts.tile([P, FT, RANK], BF16)
            nc.sync.dma_start(vsT, vt_d[:, :].rearrange("p (f r) -> p f r", r=RANK))

            x_tiles = []
            srcs = []
            for c in range(N_CHUNKS):
                xt = x_pool.tile([P, FT, CHUNK], BF16, tag="x")
                x_tiles.append(xt)
                srcs.append(xt_d[c, :, :].rearrange("p (f r) -> p f r", r=CHUNK))
            for q in range(NQ):
                nc.sync.dma_start(
                    x_tiles[0][:, q * FQ : (q + 1) * FQ, :],
                    srcs[0][:, q * FQ : (q + 1) * FQ, :],
                )
            usT = consts.tile([RANK, FEAT], BF16)
            nc.sync.dma_start(usT, ut_d[:, :])
            for q in range(NQ):
                nc.sync.dma_start(
                    x_tiles[1][:, q * FQ : (q + 1) * FQ, :],
                    srcs[1][:, q * FQ : (q + 1) * FQ, :],
                )

            # A few dummy matmuls on zeroed scratch bridge the PE from engine
            # boot until the first x quarter lands, so the HAM activity
            # window sees continuous busy-ness and lifts the clock to 2.4GHz
            # during mm1(0)'s first quarter instead of halfway through.
            warm_sb = consts.tile([P, CHUNK], BF16)
            nc.vector.memset(warm_sb, 0.0)
            # one persistent scratch PSUM bank; dummies cycle through its two
            # halves so consecutive dummies have no pool-release semaphores
            # (same-engine WAW is satisfied by program order)
            ps_w = psum_w.tile([P, 512], F32, tag="w")
            _dummy_ctr = [0]

            def emit_dummy_mm(n):
                for _ in range(n):
                    half = _dummy_ctr[0] % 2
                    _dummy_ctr[0] += 1
                    nc.tensor.matmul(
                        ps_w[:, half * 256 : (half + 1) * 256],
                        warm_sb[:, :P],
                        warm_sb[:, :256],
                        start=True,
                        stop=True,
                        skip_group_check=True,
                    )

            emit_dummy_mm(8)

            def emit_mm1_ft(c, ps_h, ft):
                nc.tensor.matmul(
                    ps_h,
                    vsT[:, ft, :],
                    x_tiles[c][:, ft, :],
                    start=(ft == 0),
                    stop=(ft == FT - 1),
                    skip_group_check=True,
                )

            def emit_mm2_rt(hT, out_sb, ci, rt, fill=0):
                # one row tile: 8 matmuls sharing the same stationary
                # hT-slice, then copies (DVE/ACT alternating) and the store.
                # `fill` dummy matmuls after the group keep the PE's HAM
                # activity up through the copy-paced drain (otherwise it
                # re-throttles to 1.2 GHz and the matmuls become the wall).
                pss = []
                for oc in range(OC):
                    ps_o = psum_o.tile([P, 512], F32, tag="po")
                    nc.tensor.matmul(
                        ps_o,
                        hT[:, rt * P : (rt + 1) * P],
                        usT[:, oc * 512 : (oc + 1) * 512],
                        start=True,
                        stop=True,
                        skip_group_check=True,
                    )
                    pss.append(ps_o)
                emit_dummy_mm(fill)
                for oc, ps_o in enumerate(pss):
                    dst = out_sb[:, rt, oc * 512 : (oc + 1) * 512]
                    if oc % 2 == 0:
                        nc.vector.tensor_copy(out=dst, in_=ps_o)
                    else:
                        nc.scalar.copy(out=dst, in_=ps_o)
                r0 = ci * CHUNK + rt * P
                nc.sync.dma_start(out_d[r0 : r0 + P, :], out_sb[:, rt, :])

            # chunk 0: mm1 over 32 feature tiles
            ps_h0 = psum_h.tile([RANK, CHUNK], F32, tag="h")
            for ft in range(FT):
                emit_mm1_ft(0, ps_h0, ft)
            hT0 = h_pool.tile([RANK, CHUNK], BF16, tag="hT")
            nc.vector.tensor_copy(out=hT0, in_=ps_h0)
            out_sb0 = out_pool.tile([P, CHUNK_TILES, FEAT], BF16, tag="out")

            # alternate mm2(0) row-tile groups with mm1(1) quarter groups:
            # mm2(0) is ready immediately (keeps the copy engines fed) while
            # mm1(1) consumes x1 quarters as they land
            ps_h1 = psum_h.tile([RANK, CHUNK], F32, tag="h")
            for q in range(NQ):
                emit_mm2_rt(hT0, out_sb0, 0, q)
                for ft in range(q * FQ, (q + 1) * FQ):
                    emit_mm1_ft(1, ps_h1, ft)
            hT1 = h_pool.tile([RANK, CHUNK], BF16, tag="hT")
            nc.vector.tensor_copy(out=hT1, in_=ps_h1)
            out_sb1 = out_pool.tile([P, CHUNK_TILES, FEAT], BF16, tag="out")
            for rt in range(CHUNK_TILES):
                emit_mm2_rt(hT1, out_sb1, 1, rt, fill=4 if rt < CHUNK_TILES - 1 else 0)
    return nc


_NC_CACHE = None


def _get_nc():
    global _NC_CACHE
    if _NC_CACHE is None:
        _NC_CACHE = build_nc()
    return _NC_CACHE


def make_in_maps(x2, U, S, V):
    xb = np.ascontiguousarray(x2, dtype=np.float32).astype(NP_BF16)
    vb = np.ascontiguousarray(V, dtype=np.float32).astype(NP_BF16)
    # vt[p, ft, r] = V[r, ft*P + p]
    vt = np.ascontiguousarray(vb.reshape(RANK, FT, P).transpose(2, 1, 0)).reshape(
        P, FT * RANK
    )
    us = np.asarray(U, dtype=np.float32) * (
        np.asarray(S, dtype=np.float32)[None, :] * SCALING
    )
    ut = np.ascontiguousarray(us.T).astype(NP_BF16)
    maps = []
    for i in range(N_CORES):
        xs = xb[i * ROWS : (i + 1) * ROWS]
        # xt[c, p, ft, r] = xs[c*CHUNK + r, ft*P + p]
        xt = np.ascontiguousarray(
            xs.reshape(N_CHUNKS, CHUNK, FT, P).transpose(0, 3, 2, 1)
        ).reshape(N_CHUNKS, P, FT * CHUNK)
        maps.append({"xt": xt, "vt": vt, "ut": ut})
    return maps


def kernel(**inputs) -> np.ndarray:
    x = np.asarray(inputs["x"])
    U = inputs["U"]
    S = inputs["S"]
    V = inputs["V"]

    b, sq, feat = x.shape
    x2 = x.reshape(b * sq, feat)

    nc = _get_nc()
    in_maps = make_in_maps(x2, U, S, V)
    res = run_bass_kernel_spmd(nc, in_maps, core_ids=list(range(N_CORES)))
    out = np.concatenate([r["out"] for r in res.results], axis=0)
    return out.astype(np.float32).reshape(b, sq, feat)


# revision 17
# speedup vs baseline: 1.2342x; 1.2079x over previous
"""LoRA layer kernel for Trainium2 (8 NeuronCores, data-parallel over rows).

Computes out = ((x @ V^T) * S) @ U^T * scaling  (scaling = alpha/rank = 1.0)
for x [4, 2048, 4096], U [4096, 32], S [32], V [32, 4096], all fp32.

Sharding: batch*seq rows (8192) split evenly across the 8 cores; the tiny
LoRA factors are replicated. All layout prep happens on the host:
  - x is cast to bf16 and pre-transposed/tiled to [chunk, p, ft, row] so the
    device reads features-on-partitions directly (no on-device transposes)
  - V is cast to bf16, pre-tiled to [p, ft, rank]
  - U is scaled by S*scaling, transposed, cast to bf16
Output is written bf16 (halves the store traffic) and upcast to fp32 on the
host; bf16 keeps max rel err ~5e-3 against the fp32 reference.

Per core (1024 rows, 2 chunks of 512):
  - input DMAs on the ACT HWDGE ring in 1 MiB ft-quarters, output DMAs on
    the SP ring, so stores interleave with loads at the SDMA engines
  - mm1: hT[32, 512] += vsT[:, ft, :]^T @ xt[:, ft, :] accumulated over the
    32 feature tiles in one PSUM bank (bf16); per-quarter DMA deps let the
    PE start after the first 1 MiB lands
  - hT copied PSUM->SBUF as bf16 (DVE)
  - mm2: per row tile, one 8-matmul group reusing the same stationary
    hT-slice (one LDWEIGHTS per 8 matmuls); chunk-0 groups are interleaved
    into mm1(1)'s feature loop so output DMA overlaps the x1 load;
    PSUM->SBUF copies split DVE/ScalarE 50/50 with bf16 downcast
  - per-row-tile 1 MiB DMA stores issued right after each group's copies
Roofline: ~16.4 MiB HBM traffic per core at ~360-425 GB/s => ~42-47 us;
PE ~33 us hidden under DMA. No collectives needed.
"""

import sys

for _p in ("/root/.axon_site/_ro/trn_rl_repo", "/opt/trn_rl_repo"):
    if _p not in sys.path:
        sys.path.append(_p)

import ml_dtypes
import numpy as np

import concourse.bass as bass
from concourse import mybir
from concourse.bass_utils import run_bass_kernel_spmd
from concourse.tile import TileContext

F32 = mybir.dt.float32
BF16 = mybir.dt.bfloat16
NP_BF16 = ml_dtypes.bfloat16

P = 128
ROWS = 1024  # per-core row shard
FEAT = 4096
RANK = 32
SCALING = 1.0  # alpha / max_rank = 32 / 32
FT = FEAT // P  # 32 feature tiles
CHUNK = 512  # rows per pipeline chunk
CHUNK_TILES = CHUNK // P  # 4
N_CHUNKS = ROWS // CHUNK  # 2
NQ = 4  # input DMA quarters per chunk
FQ = FT // NQ  # 8 feature tiles per quarter
OC = FEAT // 512  # 8 output column chunks per row tile
N_CORES = 8


def _split_multiwaits(nc) -> None:
    # Workaround for this container's walrus: engine instructions with >=2
    # sem waits fail codegen ("Too many sync wait commands"). Hoist all but
    # the last wait onto single-wait NoOps inserted just before, same engine.
    for f in nc.m.functions:
        for bb in f.blocks:
            out = []
            changed = False
            for inst in bb.instructions:
                si = inst.sync_info
                waits = list(si.on_wait) if (si is not None and si.on_wait) else []
                if len(waits) > 1:
                    changed = True
                    for w in waits[:-1]:
                        nop = mybir.InstNoOp(name=f"splitw-{nc.next_id()}")
                        nop.engine = inst.engine
                        nop.sync_info = mybir.SyncInfo(on_wait=[w], on_update=[])
                        nc.register_instruction(nop)
                        out.append(nop)
                    si.on_wait = [waits[-1]]
                out.append(inst)
            if changed:
                bb.instructions = out


class _PatchedTileContext(TileContext):
    def _drain_and_barrier(self, tick_clock, wait_clock):
        super()._drain_and_barrier(tick_clock, wait_clock)
        _split_multiwaits(self.nc)


def build_nc() -> bass.Bass:
    nc = bass.Bass(trn_type="TRN2", target_bir_lowering=False, name="lora")
    # xt host layout: [chunk, p, ft, row-in-chunk]; ft-quarter slices are
    # 8 KiB-per-partition contiguous DMAs
    xt_d = nc.dram_tensor("xt", [N_CHUNKS, P, FT * CHUNK], BF16, kind="ExternalInput")
    vt_d = nc.dram_tensor("vt", [P, FT * RANK], BF16, kind="ExternalInput")
    ut_d = nc.dram_tensor("ut", [RANK, FEAT], BF16, kind="ExternalInput")
    out_d = nc.dram_tensor("out", [ROWS, FEAT], BF16, kind="ExternalOutput")

    with _PatchedTileContext(nc) as tc:
        with (
            tc.tile_pool(name="consts", bufs=1) as consts,
            tc.tile_pool(name="xin", bufs=N_CHUNKS) as x_pool,
            tc.tile_pool(name="hts", bufs=2) as h_pool,
            tc.tile_pool(name="outs", bufs=2) as out_pool,
            tc.tile_pool(name="ps_h", bufs=2, space="PSUM") as psum_h,
            tc.tile_pool(name="ps_o", bufs=5, space="PSUM") as psum_o,
            tc.tile_pool(name="ps_w", bufs=1, space="PSUM") as psum_w,
        ):
            # All DMAs ride the SP (sync) HWDGE ring: the FIFO gives input
            # loads strict priority over output stores, which is what we want
            # — the back half is production(copy)-limited anyway, and early
            # input completion unblocks mm1(1). Issue order = need order:
            # vt, x0 quarters, ut, x1 quarters, stores as produced.
            vsT = consts.tile([P, FT, RANK], BF16)
            nc.sync.dma_start(vsT, vt_d[:, :].rearrange("p (f r) -> p f r", r=RANK))

            x_tiles = []
            srcs = []
            for c in range(N_CHUNKS):
                xt = x_pool.tile([P, FT, CHUNK], BF16, tag="x")
                x_tiles.append(xt)
                srcs.append(xt_d[c, :, :].rearrange("p (f r) -> p f r", r=CHUNK))
            for q in range(NQ):
                nc.sync.dma_start(
                    x_tiles[0][:, q * FQ : (q + 1) * FQ, :],
                    srcs[0][:, q * FQ : (q + 1) * FQ, :],
                )
            usT = consts.tile([RANK, FEAT], BF16)
            nc.sync.dma_start(usT, ut_d[:, :])
            for q in range(NQ):
                nc.sync.dma_start(
                    x_tiles[1][:, q * FQ : (q + 1) * FQ, :],
                    srcs[1][:, q * FQ : (q + 1) * FQ, :],
                )

            # A few dummy matmuls on zeroed scratch bridge the PE from engine
            # boot until the first x quarter lands, so the HAM activity
            # window sees continuous busy-ness and lifts the clock to 2.4GHz
            # during mm1(0)'s first quarter instead of halfway through.
            warm_sb = consts.tile([P, CHUNK], BF16)
            nc.vector.memset(warm_sb, 0.0)
            # one persistent scratch PSUM bank; dummies cycle through its two
            # halves so consecutive dummies have no pool-release semaphores
            # (same-engine WAW is satisfied by program order)
            ps_w = psum_w.tile([P, 512], F32, tag="w")
            _dummy_ctr = [0]

            def emit_dummy_mm(n):
                for _ in range(n):
                    half = _dummy_ctr[0] % 2
                    _dummy_ctr[0] += 1
                    nc.tensor.matmul(
                        ps_w[:, half * 256 : (half + 1) * 256],
                        warm_sb[:, :P],
                        warm_sb[:, :256],
                        start=True,
                        stop=True,
                        skip_group_check=True,
                    )

            emit_dummy_mm(8)

            def emit_mm1_ft(c, ps_h, ft):
                nc.tensor.matmul(
                    ps_h,
                    vsT[:, ft, :],
                    x_tiles[c][:, ft, :],
                    start=(ft == 0),
                    stop=(ft == FT - 1),
                    skip_group_check=True,
                )

            def emit_mm2_rt(hT, out_sb, ci, rt, fill=0):
                # one row tile: 8 matmuls sharing the same stationary
                # hT-slice, then copies (DVE/ACT alternating) and the store.
                # `fill` dummy matmuls after the group keep the PE's HAM
                # activity up through the copy-paced drain (otherwise it
                # re-throttles to 1.2 GHz and the matmuls become the wall).
                # The dummies read hT so the scheduler cannot hoist them out
                # of the drain (they anchor to real data flow).
                pss = []
                for oc in range(OC):
                    ps_o = psum_o.tile([P, 512], F32, tag="po")
                    nc.tensor.matmul(
                        ps_o,
                        hT[:, rt * P : (rt + 1) * P],
                        usT[:, oc * 512 : (oc + 1) * 512],
                        start=True,
                        stop=True,
                        skip_group_check=True,
                    )
                    pss.append(ps_o)
                for k in range(fill):
                    half = _dummy_ctr[0] % 2
                    _dummy_ctr[0] += 1
                    nc.tensor.matmul(
                        ps_w[:, half * 256 : (half + 1) * 256],
                        hT[:, :P],
                        hT[:, :256],
                        start=True,
                        stop=True,
                        skip_group_check=True,
                    )
                for oc, ps_o in enumerate(pss):
                    dst = out_sb[:, rt, oc * 512 : (oc + 1) * 512]
                    if oc % 2 == 0:
                        nc.vector.tensor_copy(out=dst, in_=ps_o)
                    else:
                        nc.scalar.copy(out=dst, in_=ps_o)
                r0 = ci * CHUNK + rt * P
                nc.sync.dma_start(out_d[r0 : r0 + P, :], out_sb[:, rt, :])

            # chunk 0: mm1 over 32 feature tiles
            ps_h0 = psum_h.tile([RANK, CHUNK], F32, tag="h")
            for ft in range(FT):
                emit_mm1_ft(0, ps_h0, ft)
            hT0 = h_pool.tile([RANK, CHUNK], BF16, tag="hT")
            nc.vector.tensor_copy(out=hT0, in_=ps_h0)
            out_sb0 = out_pool.tile([P, CHUNK_TILES, FEAT], BF16, tag="out")

            # alternate mm2(0) row-tile groups with mm1(1) quarter groups:
            # mm2(0) is ready immediately (keeps the copy engines fed) while
            # mm1(1) consumes x1 quarters as they land
            ps_h1 = psum_h.tile([RANK, CHUNK], F32, tag="h")
            for q in range(NQ):
                emit_mm2_rt(hT0, out_sb0, 0, q)
                for ft in range(q * FQ, (q + 1) * FQ):
                    emit_mm1_ft(1, ps_h1, ft)
            hT1 = h_pool.tile([RANK, CHUNK], BF16, tag="hT")
            nc.vector.tensor_copy(out=hT1, in_=ps_h1)
            out_sb1 = out_pool.tile([P, CHUNK_TILES, FEAT], BF16, tag="out")
            for rt in range(CHUNK_TILES):
                emit_mm2_rt(hT1, out_sb1, 1, rt, fill=4 if rt < CHUNK_TILES - 1 else 0)
    return nc


_NC_CACHE = None


def _get_nc():
    global _NC_CACHE
    if _NC_CACHE is None:
        _NC_CACHE = build_nc()
    return _NC_CACHE


def make_in_maps(x2, U, S, V):
    xb = np.ascontiguousarray(x2, dtype=np.float32).astype(NP_BF16)
    vb = np.ascontiguousarray(V, dtype=np.float32).astype(NP_BF16)
    # vt[p, ft, r] = V[r, ft*P + p]
    vt = np.ascontiguousarray(vb.reshape(RANK, FT, P).transpose(2, 1, 0)).reshape(
        P, FT * RANK
    )
    us = np.asarray(U, dtype=np.float32) * (
        np.asarray(S, dtype=np.float32)[None, :] * SCALING
    )
    ut = np.ascontiguousarray(us.T).astype(NP_BF16)
    maps = []
    for i in range(N_CORES):
        xs = xb[i * ROWS : (i + 1) * ROWS]
        # xt[c, p, ft, r] = xs[c*CHUNK + r, ft*P + p]
        xt = np.ascontiguousarray(
            xs.reshape(N_CHUNKS, CHUNK, FT, P).transpose(0, 3, 2, 1)
        ).reshape(N_CHUNKS, P, FT * CHUNK)
        maps.append({"xt": xt, "vt": vt, "ut": ut})
    return maps


def kernel(**inputs) -> np.ndarray:
    x = np.asarray(inputs["x"])
    U = inputs["U"]
    S = inputs["S"]
    V = inputs["V"]

    b, sq, feat = x.shape
    x2 = x.reshape(b * sq, feat)

    nc = _get_nc()
    in_maps = make_in_maps(x2, U, S, V)
    res = run_bass_kernel_spmd(nc, in_maps, core_ids=list(range(N_CORES)))
    out = np.concatenate([r["out"] for r in res.results], axis=0)
    return out.astype(np.float32).reshape(b, sq, feat)
